# revision 5
# baseline (speedup 1.0000x reference)
"""Trainium2 Bass kernel for nn_MinEuclideanDistBlock.

Math (reference):
  x: (B=64, C=3, L=2048), shapelets: (C=3, N=256, S=64)
  W = L - S + 1 = 1985 sliding windows
  d2[b,c,w,n] = |win|^2 + |shp|^2 - 2 win.shp    (win = x[b,c,w:w+S])
  d = sqrt(max(d2, 0));  out[b,0,n] = min_w sum_c d[b,c,w,n]

Device strategy (per core, batch-sharded B/8 = 8 batches per core):
  - T matrices per (b,c): rows 0..63 = x-shift windows (T[s,w]=x[w+s]),
    row 64/65 = win_sq hi/lo (bf16 split of the f32 value), row 66 = ones.
    lhsT rows: 0..63 = -2*shapelets^T, 64/65 = 1.0, 66 = shp_sq (bf16).
    psum[n,w] = d2 directly -- no ACT bias, activations are uniform sqrt.
  - Fully chunked pipeline at 512 cols (PSUM bank width): 8-deep psum
    ring of [128,512] tiles. Measured on this backend: a matmul+ACT
    chunk through an 8-ring runs ~293ns vs ~1596ns for the 4-chunk
    matmul -> 2-ring [128,2048] -> wide-ACT structure (PE/ACT do NOT
    overlap through a 2-slot ring; separate small tiles pipeline).
    Chunk tiles are SEPARATE tiles (shared wide dst serializes).
  - T tiles loaded once per NEFF (hoisted) in 5 SWDGE groups (3,3,6,6,6
    bc) so first matmuls start ~4us in while the rest stream.
  - Back half per (b,nt,chunk): DVE adds (bf16 4x mode, ~270ns/chunk),
    chunk min via tensor_scalar accum (trash out is dead -> compiler
    drops the write), then a final [128,4] min into the result column.
    tensor_tensor_reduce would fuse add+min but crashes the runtime
    (NRT_EXEC_UNIT_UNRECOVERABLE -- verified).
  - Results accumulate into one [128, 16] SBUF tile, written out in two
    DMAs (first half early, tail half at the end).
"""

import numpy as np

S = 64
NSH = 256
C = 3
B = 64
L = 2048
W = L - S + 1  # 1985
NCORES = 8
BPC = B // NCORES  # 8
NT = 2  # shapelet tiles of 128
KR = S + 3  # lhsT/T rows: 64 shifts + wsq_hi + wsq_lo + ones
CHUNKS = [(0, 512), (512, 512), (1024, 512), (1536, W - 1536)]
NCH = len(CHUNKS)
GROUPS = [3, 3, 6, 6, 6]  # bc counts per hoisted T-load group

# tunable: chunk indices whose adds run on gpsimd (Pool) instead of DVE
POOL_CHUNKS_C1 = ()
POOL_CHUNKS_C2 = ()
LDW_DEDUP = True

_cache = {}


def _build_nc(reps=1, ablate=()):
    import concourse.bass as bass
    import concourse.bacc as bacc
    import concourse.mybir as mybir
    import concourse.tile as tile

    f32 = mybir.dt.float32
    bf16 = mybir.dt.bfloat16
    AF = mybir.ActivationFunctionType

    nc = bacc.Bacc()
    xs = nc.dram_tensor("xs", [BPC, C, L], bf16, kind="ExternalInput")
    wts = nc.dram_tensor("wts", [KR, C * NT * 128], bf16, kind="ExternalInput")
    wsq = nc.dram_tensor("wsq", [BPC * C, 3, L], bf16, kind="ExternalInput")
    out = nc.dram_tensor("out", [128, BPC * NT], f32, kind="ExternalOutput")

    with tile.TileContext(nc) as tc:
        with (
            tc.tile_pool(name="consts", bufs=1) as consts,
            tc.tile_pool(name="tpool", bufs=1) as tpool,
            tc.tile_pool(name="psumc", bufs=8, space="PSUM") as psumc,
            tc.tile_pool(name="accp", bufs=3) as accp,
            tc.tile_pool(name="tmpp", bufs=3) as tmpp,
            tc.tile_pool(name="minvp", bufs=4) as minvp,
        ):
            # ---- prewarm the sqrt activation table while DMAs stream ----
            warm = consts.tile([128, 1], bf16, name="warm")
            nc.gpsimd.memset(warm, 1.0)
            nc.scalar.activation(warm, warm, AF.Sqrt)

            # ---- constants: single merged weights DMA ----
            w_all = consts.tile([KR, C * NT * 128], bf16)
            nc.sync.dma_start(out=w_all, in_=wts[:, :])

            # ---- hoisted T loads: content is invariant across reps ----
            talls = []  # (Tall, bc0, gb)
            bc0 = 0
            for g, gb in enumerate(GROUPS):
                Tall = tpool.tile([KR, gb, L], bf16, name=f"Tall{g}")
                if "nodma" not in ablate:
                    base = xs[bc0 // C, bc0 % C, :]
                    apov = bass.AP(
                        tensor=base.tensor,
                        offset=base.offset,
                        ap=[[1, S], [L, gb], [1, W]],
                    )
                    nc.gpsimd.dma_start(out=Tall[0:S, 0:gb, 0:W], in_=apov)
                    wbase = wsq[bc0, 0, :]
                    apwq = bass.AP(
                        tensor=wbase.tensor,
                        offset=wbase.offset,
                        ap=[[L, 3], [3 * L, gb], [1, W]],
                    )
                    nc.gpsimd.dma_start(out=Tall[S : S + 3, 0:gb, 0:W], in_=apwq)
                talls.append((Tall, bc0, gb))
                bc0 += gb
            for _rep in range(reps):
                _body(nc, tc, bass, mybir, talls, psumc, accp, tmpp,
                      minvp, out, w_all, ablate)
    if LDW_DEDUP:
        _dedup_ldweights(nc)
    nc.compile()
    return nc


def _dedup_ldweights(nc):
    """Drop Ldweights that reload the exact weights already resident in the
    PE array (same source AP as the previous Ldweights, nothing between
    them that could clobber the array). Only duplicates with no semaphore
    waits/updates are removed."""
    removed = 0
    for blk in nc.m.functions[0].blocks:
        prev_sig = None
        keep = []
        for inst in blk.instructions:
            if inst.opcode == "Ldweights":
                sig = str(inst.ins[0])
                si = inst.sync_info
                clean = si is None or (
                    len(si.on_wait) == 0 and len(si.on_update) == 0
                )
                if sig == prev_sig and clean:
                    removed += 1
                    continue
                prev_sig = sig
            keep.append(inst)
        if removed:
            blk.instructions = keep
    return removed


def _body(nc, tc, bass, mybir, talls, psumc, accp, tmpp, minvp,
          out, w_all, ablate=()):
    f32 = mybir.dt.float32
    bf16 = mybir.dt.bfloat16
    AF = mybir.ActivationFunctionType
    ALU = mybir.AluOpType

    mm_on = "nomm" not in ablate
    act_on = mm_on and "noact" not in ablate
    add_on = act_on and "noadd" not in ablate
    red_on = add_on and "nored" not in ablate

    accs = {}
    minv_all = minvp.tile([128, BPC * NT], f32, name="minv_all")
    half_cols = BPC * NT // 2
    half_done = False
    for Tall, bc0, gb in talls:
        # (c, nt)-major within the group: consecutive matmul sets share
        # lhsT, so the Ldweights dedup collapses them to one load per set
        order = [
            (j, nt)
            for c in range(C)
            for nt in range(NT)
            for j in range(gb)
            if (bc0 + j) % C == c
        ]
        for j, nt in order:
            bc = bc0 + j
            b, c = bc // C, bc % C
            idx = c * NT + nt
            lhsT = w_all[:, idx * 128 : (idx + 1) * 128]
            pss = []
            if mm_on:
                for w0, wl in CHUNKS:
                    ps = psumc.tile([128, 512], f32, name="ps")
                    nc.tensor.matmul(
                        ps[:, 0:wl],
                        lhsT=lhsT,
                        rhs=Tall[:, j, w0 : w0 + wl],
                        start=True,
                        stop=True,
                    )
                    pss.append(ps)
            if not act_on:
                continue
            if c == 0:
                unit = []
                for k, (w0, wl) in enumerate(CHUNKS):
                    acc = accp.tile([128, 512], bf16, name=f"acc{nt}{k}")
                    nc.scalar.activation(acc[:, 0:wl], pss[k][:, 0:wl], AF.Sqrt)
                    unit.append(acc)
                accs[(b, nt)] = unit
            elif c == 1:
                unit = accs[(b, nt)]
                for k, (w0, wl) in enumerate(CHUNKS):
                    tmp = tmpp.tile([128, 512], bf16, name=f"tmp{k}")
                    nc.scalar.activation(tmp[:, 0:wl], pss[k][:, 0:wl], AF.Sqrt)
                    if add_on:
                        eng = nc.gpsimd if k in POOL_CHUNKS_C1 else nc.vector
                        eng.tensor_add(
                            unit[k][:, 0:wl], unit[k][:, 0:wl], tmp[:, 0:wl]
                        )
            else:
                unit = accs[(b, nt)]
                col = b * NT + nt
                mc = minvp.tile([128, NCH], f32, name="mc")
                for k, (w0, wl) in enumerate(CHUNKS):
                    tmp = tmpp.tile([128, 512], bf16, name=f"tmp{k}")
                    nc.scalar.activation(tmp[:, 0:wl], pss[k][:, 0:wl], AF.Sqrt)
                    if add_on:
                        scr = tmpp.tile([128, 512], bf16, name=f"scr{k}")
                        eng = nc.gpsimd if k in POOL_CHUNKS_C2 else nc.vector
                        eng.tensor_add(
                            scr[:, 0:wl], unit[k][:, 0:wl], tmp[:, 0:wl]
                        )
                    if red_on:
                        trash = tmpp.tile([128, 512], bf16, name=f"trash{k}")
                        nc.vector.tensor_scalar(
                            out=trash[:, 0:wl], in0=scr[:, 0:wl], scalar1=0.0,
                            scalar2=None, op0=ALU.add, op1=ALU.min,
                            accum_out=mc[:, k : k + 1],
                        )
                if red_on:
                    trash2 = minvp.tile([128, NCH], f32, name="trash2")
                    nc.vector.tensor_scalar(
                        out=trash2, in0=mc, scalar1=0.0,
                        scalar2=None, op0=ALU.add, op1=ALU.min,
                        accum_out=minv_all[:, col : col + 1],
                    )
                else:
                    nc.vector.memset(minv_all[:, col : col + 1], 0.0)
        # first half of the output leaves as soon as b0..b3 are reduced
        if not half_done and bc0 + gb >= (BPC // 2) * C:
            nc.sync.dma_start(
                out=out[:, 0:half_cols], in_=minv_all[:, 0:half_cols]
            )
            half_done = True
    nc.sync.dma_start(
        out=out[:, half_cols:], in_=minv_all[:, half_cols:]
    )


def _get_nc():
    if "nc" not in _cache:
        _cache["nc"] = _build_nc()
    return _cache["nc"]


def _prep_inputs(x, shapelets):
    import ml_dtypes

    bf16 = ml_dtypes.bfloat16
    x = np.ascontiguousarray(np.asarray(x), dtype=np.float32)
    sh = np.asarray(shapelets, dtype=np.float32)
    # round shapelets to bf16 once; all derived quantities use the rounded
    # values so d2 stays an exact squared distance of the rounded vectors
    shb = sh.astype(bf16).astype(np.float32)
    shT = np.transpose(shb, (0, 2, 1))  # (C, S, N)
    ssq = np.sum(shb * shb, axis=2)  # (C, N)
    wts = np.empty((KR, C * NT * 128), np.float32)
    for c in range(C):
        for nt in range(NT):
            i0 = (c * NT + nt) * 128
            wts[:S, i0 : i0 + 128] = -2.0 * shT[c, :, nt * 128 : (nt + 1) * 128]
            wts[S + 2, i0 : i0 + 128] = ssq[c, nt * 128 : (nt + 1) * 128]
    wts[S : S + 2, :] = 1.0
    wts_b = np.ascontiguousarray(wts.astype(bf16))
    xb = x.astype(bf16)
    # win_sq from the bf16-rounded x (what the device matmul sees), split
    # hi/lo so the bf16 pair reconstructs the f32 value in the PE f32 accum
    xf = xb.astype(np.float32)
    xsq = xf * xf
    cums = np.concatenate(
        [np.zeros((B, C, 1), np.float32), np.cumsum(xsq, axis=2)], axis=2
    )
    winsq_full = np.zeros((B, C, L), np.float32)
    winsq_full[:, :, :W] = cums[:, :, S : L + 1] - cums[:, :, 0:W]
    wsq_hi = winsq_full.astype(bf16)
    wsq_lo = (winsq_full - wsq_hi.astype(np.float32)).astype(bf16)
    wsq3 = np.empty((B, C, 3, L), bf16)
    wsq3[:, :, 0] = wsq_hi
    wsq3[:, :, 1] = wsq_lo
    wsq3[:, :, 2] = np.float32(1.0)
    in_maps = [
        {
            "xs": np.ascontiguousarray(xb[k * BPC : (k + 1) * BPC]),
            "wts": wts_b,
            "wsq": np.ascontiguousarray(
                wsq3[k * BPC : (k + 1) * BPC].reshape(BPC * C, 3, L)
            ),
        }
        for k in range(NCORES)
    ]
    return in_maps


def _gather(results):
    outs = []
    for r in results:
        o = np.asarray(r["out"]).reshape(128, BPC, NT)  # [n128, b, nt]
        outs.append(np.transpose(o, (1, 2, 0)).reshape(BPC, NSH))
    full = np.concatenate(outs, axis=0)  # (64, 256)
    return np.ascontiguousarray(full[:, None, :]).astype(np.float32)  # (64, 1, 256)


def kernel(x, shapelets):
    from concourse.bass_utils import run_bass_kernel_spmd

    nc = _get_nc()
    in_maps = _prep_inputs(x, shapelets)
    res = run_bass_kernel_spmd(nc, in_maps, core_ids=list(range(NCORES)))
    return _gather(res.results)


def kernel_traced(x, shapelets):
    """Like kernel() but requests an NTFF trace; returns (out, BassKernelResults)."""
    from concourse.bass_utils import run_bass_kernel_spmd

    nc = _get_nc()
    in_maps = _prep_inputs(x, shapelets)
    res = run_bass_kernel_spmd(nc, in_maps, core_ids=list(range(NCORES)), trace=True)
    return _gather(res.results), res
